# revision 1
# baseline (speedup 1.0000x reference)
"""Trainium2 Bass kernel: batched forward kinematics (nn_DiffKin).

Computes, for each batch element b and frame n:
    W[b, n] = prod_{i<=n} ( O_i @ M_i(angle_i(b)) )        (4x4 transforms)
where M_i is a revolute rotation / prismatic translation about a fixed axis.

Strategy (pure data-parallel across 8 NeuronCores, batch-major layout):
  * Host folds origins/axes/joint-types/mimic into per-frame constant
    3x4 (or 4x4) tables A' = A + C, B, C with
        L_n(b) = A'_n + u_n(b) * B_n + w_n(b) * C_n,
        u = sin(m*theta+o)  (or  m*theta+o  for prismatic),
        w = -cos(m*theta+o) = sin(m*theta+o - pi/2).
    so the only per-batch device work is two ScalarE Sin evaluations per
    frame plus elementwise tensor ops.
  * SBUF layout: partition p holds batch elements b = p*Q + q (q inner,
    contiguous) so the final DMA writes long contiguous HBM runs.
  * Per frame: GpSimd builds L (2 muls + add), VectorE runs the affine
    chain product (5 fused-broadcast tensor ops), ScalarE feeds coefficients.
  * Output staged in SBUF chunks of C frames, DMA'd out overlapped.

The program is specialized at trace time on the structural inputs (indices,
types, axes norms); batch data flows through DRAM tensors.

Performance state (TimelineSim model / HW differential measurement):
  total ~383 us modeled (~560-680 us measured incl. sequencer overheads);
  VectorE ~304 us busy == GpSimd ~303 us busy (balanced floor of this
  all-elementwise architecture), DMA ~105 us, ScalarE ~67 us, PE idle.
Next step if iterating further (worked out, not landed): build the L
matrices on the idle TensorEngine — per 128-batch column qq, one PE
transpose of the coefficient tile uw[:, qq, :] ([128, ~124] -> PSUM),
evacuate to SBUF, then 2 fp32 matmuls against a host-built block table
[1+2F, 12F] (ones-row + u/w rows per frame block; PSUM-accumulate adds
the A' term via the ones-row) producing L for all frames of that column
batch-major in PSUM. Evacuate as fp16 into a [128, nf, 12, q] SBUF
buffer (98 KB/partition; fp32 doesn't fit). This removes the GpSimd
L-build entirely, letting the chain products split DVE/GpSimd ~2:1
(projected ~195 us each, ~200-210 us total). Precision VERIFIED by
simulation: fp16-L with fp32 state gives rel_l2 = 8.9e-4 end-to-end
(maxabs/scale 1.3e-3) - safe. float32r matmul dtype (1 cyc/row at
N>=256 vs 4 for fp32) is worth testing for the PE step.
"""

import os
import sys

import numpy as np

for _p in ("/opt/trn_rl_repo", "/root/.axon_site/_ro/trn_rl_repo"):
    if os.path.isdir(_p) and _p not in sys.path:
        sys.path.append(_p)

import concourse.bass as bass  # noqa: E402
import concourse.tile as tile  # noqa: E402
from concourse import bacc, mybir  # noqa: E402
from concourse.bass_utils import run_bass_kernel_spmd  # noqa: E402

F32 = mybir.dt.float32
AF = mybir.ActivationFunctionType

N_CORES = 8
P = 128  # SBUF partitions
CHUNK = 8  # frames per output staging chunk

# module-level stash for test harness introspection
last_results = None
last_tables_rep = None
_program_cache = {}


# --------------------------------------------------------------------------
# Host-side specialization
# --------------------------------------------------------------------------

def _skew(a):
    x, y, z = a
    return np.array([[0.0, -z, y], [z, 0.0, -x], [-y, x, 0.0]], dtype=np.float64)


def _frame_specs(all_axes, all_origins, mimic_multipliers, mimic_offsets,
                 ctrlable_indices, mimic_dst_indices, mimic_src_indices,
                 joint_types):
    """Fold structural inputs into per-frame specs + constant tables.

    Returns (affine, frames, tables):
      affine  : True if all origin bottom rows are [0,0,0,1] (3x4 chain math)
      frames  : list of dicts per frame:
                  kind: 'rev' | 'pri' | 'const'
                  src  : source column into joint_angles (var kinds)
                  mult, off : effective angle transform (var kinds)
      tables  : np.float32 [NF, 4, 16]  (slots: A', B, C, scalars)
    """
    axes = np.asarray(all_axes, dtype=np.float64)
    origins = np.asarray(all_origins, dtype=np.float64)
    nf = origins.shape[0]
    types = np.asarray(joint_types).astype(np.int64)
    ctrl = np.asarray(ctrlable_indices).astype(np.int64)
    mdst = np.asarray(mimic_dst_indices).astype(np.int64)
    msrc = np.asarray(mimic_src_indices).astype(np.int64)
    mmul = np.asarray(mimic_multipliers, dtype=np.float64)
    moff = np.asarray(mimic_offsets, dtype=np.float64)

    bottom = origins[:, 3, :]
    affine = bool(np.all(np.abs(bottom - np.array([0.0, 0.0, 0.0, 1.0])) < 1e-6))
    ni = 3 if affine else 4

    # per-frame angle source: angle_n(b) = mult * theta[b, src] + off
    # (src=None -> constant angle `off`)
    src = [None] * nf
    mult = [0.0] * nf
    off = [0.0] * nf
    for j, ci in enumerate(ctrl):
        src[int(ci)] = j
        mult[int(ci)] = 1.0
        off[int(ci)] = 0.0
    # mimic reads post-ctrl pre-mimic values
    pre_src = list(src)
    pre_mult = list(mult)
    pre_off = list(off)
    for d, s, m, o in zip(mdst, msrc, mmul, moff):
        d, s = int(d), int(s)
        if pre_src[s] is not None:
            src[d] = pre_src[s]
            mult[d] = float(m) * pre_mult[s]
            off[d] = float(m) * pre_off[s] + float(o)
        else:
            src[d] = None
            mult[d] = 0.0
            off[d] = float(o)  # constant angle

    frames = []
    tables = np.zeros((nf, 4, 16), dtype=np.float64)

    def put(slot, n, mat):  # mat is (ni, 4)
        tables[n, slot, : ni * 4] = mat.reshape(-1)

    for n in range(nf):
        O4 = origins[n]
        A = O4[:ni, :].copy()
        t = int(types[n])
        if t == 1:  # revolute
            r = float(np.linalg.norm(axes[n]))
            if r < 1e-20:
                t = 0  # degenerate axis -> identity rotation
            else:
                K4 = np.zeros((4, 4))
                K4[:3, :3] = _skew(axes[n] / r)
                B = (O4 @ K4)[:ni, :]
                C = (O4 @ K4 @ K4)[:ni, :]
                if src[n] is None:
                    a = r * off[n]
                    put(0, n, A + np.sin(a) * B + (1.0 - np.cos(a)) * C)
                    frames.append(dict(kind="const"))
                else:
                    # L = A' + sin(x)*B + cos(x)*(-C)  with A' = A + C
                    put(0, n, A + C)
                    put(1, n, B)
                    put(2, n, -C)
                    tables[n, 3, 2] = np.pi / 2.0
                    fr = dict(kind="rev", src=src[n],
                              mult=r * mult[n], off=r * off[n])
                    if affine:
                        fr["tcol"] = tuple(float(tables[n, 0, k * 4 + 3])
                                           for k in range(3))
                    frames.append(fr)
                continue
        if t == 2:  # prismatic (raw, unnormalized axis)
            T4 = np.zeros((4, 4))
            T4[:3, 3] = axes[n]
            B = (O4 @ T4)[:ni, :]
            if src[n] is None:
                put(0, n, A + off[n] * B)
                frames.append(dict(kind="const"))
            else:
                put(0, n, A)
                put(1, n, B)
                frames.append(dict(kind="pri", src=src[n],
                                   mult=mult[n], off=off[n]))
            continue
        # fixed / degenerate
        put(0, n, A)
        frames.append(dict(kind="const"))

    if affine:
        for n, fr in enumerate(frames):
            if fr["kind"] == "const":
                fr["tcol"] = tuple(float(tables[n, 0, k * 4 + 3])
                                   for k in range(3))

    return affine, frames, tables.astype(np.float32)


# --------------------------------------------------------------------------
# Device program
# --------------------------------------------------------------------------

def _build_program(b_core, dof, nf, affine, frames):
    """Builds the Bass/Tile program. Returns compiled Bacc."""
    assert b_core % P == 0
    q = b_core // P  # batch elements per partition (inner, contiguous)
    ni = 3 if affine else 4  # state rows
    nk = ni  # contraction extent in the chain product
    nchunks = (nf + CHUNK - 1) // CHUNK
    assert nf % CHUNK == 0

    nc = bacc.Bacc("TRN2", target_bir_lowering=False, debug=False)

    theta_d = nc.dram_tensor("theta", [b_core, dof], F32, kind="ExternalInput").ap()
    tables_d = nc.dram_tensor("tables", [P, nf, 4, 16], F32,
                              kind="ExternalInput").ap()
    out_d = nc.dram_tensor("out", [b_core, nf * 16], F32,
                           kind="ExternalOutput").ap()

    theta_v = theta_d.rearrange("(p q) d -> p q d", p=P)
    out_v = out_d.rearrange("(p q) (n e) -> p q n e", p=P, e=16)

    from contextlib import ExitStack

    reps = int(os.environ.get("FK_REPS", "1"))

    with tile.TileContext(nc) as tc, ExitStack() as ctx:
        pool = ctx.enter_context(tc.tile_pool(name="persist", bufs=1))
        lpool = ctx.enter_context(tc.tile_pool(name="lpool", bufs=4))
        mpool = ctx.enter_context(tc.tile_pool(name="mpool", bufs=4))

        theta_t = pool.tile([P, q, dof], F32)
        nc.sync.dma_start(theta_t[:], theta_v)

        tables_t = pool.tile([P, nf, 4, 16], F32)
        nc.sync.dma_start(tables_t[:], tables_d)

        u_t = pool.tile([P, nf, q], F32, tag="u_t")
        w_t = pool.tile([P, nf, q], F32, tag="w_t")

        # staging buffers (manual double buffer)
        stags = [pool.tile([P, q, CHUNK, 16], F32, tag=f"stag{i}",
                           name=f"stag{i}") for i in range(2)]
        if affine:
            for st in stags:
                nc.vector.memset(st[:, :, :, 12:15], 0.0)
                nc.vector.memset(st[:, :, :, 15], 1.0)

        for _rep in range(reps):
            # ---- coefficient planes ------------------------------------------
            # x = clamp(mult*theta+off, [-pi, pi]); u = sin(x); w = cos(x)
            # (cos computed as Sin(pi/2 - |x|) since the ScalarE Sin LUT only
            # accepts [-pi, pi]).
            pi = float(np.pi)
            op = mybir.AluOpType
            xpool = ctx.enter_context(tc.tile_pool(name="xpool", bufs=3))
            for n, fr in enumerate(frames):
                if fr["kind"] == "rev":
                    src_ap = theta_t[:, :, fr["src"]]
                    x_c = xpool.tile([P, q], F32, tag="xc")
                    nc.vector.tensor_scalar(x_c[:], src_ap, fr["mult"], fr["off"],
                                            op0=op.mult, op1=op.add)
                    nc.vector.tensor_scalar(x_c[:], x_c[:], pi, -pi,
                                            op0=op.min, op1=op.max)
                    nc.scalar.activation(u_t[:, n, :], x_c[:], AF.Sin)
                    a_x = xpool.tile([P, q], F32, tag="ax")
                    nc.scalar.activation(a_x[:], x_c[:], AF.Abs)
                    nc.scalar.activation(w_t[:, n, :], a_x[:], AF.Sin,
                                         bias=tables_t[:, n, 3, 2:3],
                                         scale=-1.0)
                elif fr["kind"] == "pri":
                    src_ap = theta_t[:, :, fr["src"]]
                    nc.vector.tensor_scalar(u_t[:, n, :], src_ap,
                                            fr["mult"], fr["off"],
                                            op0=op.mult, op1=op.add)

            # ---- helpers ------------------------------------------------------
            def tab(n, slot):
                # [P, 4(k), 4(j)] view of one table matrix
                return tables_t[:, n, slot, :].rearrange("p (k j) -> p k j", j=4)

            def tab_b(n, slot, nk_, nj_):
                # broadcast to [P, nk_, nj_, q] (steps 0 on q)
                a = tab(n, slot)[:, :nk_, :nj_]
                return a.unsqueeze(3).broadcast_to([P, nk_, nj_, q])

            def stag_view(ci, c):
                # [P, 4(i), 4(j), q] of staged frame transform
                return stags[ci][:, :, c, :] \
                    .rearrange("p q (i j) -> p q i j", j=4).transpose([0, 2, 3, 1])

            # ---- per-frame scan ----------------------------------------------
            prev = None  # (chunk_tile_idx, c)
            for n, fr in enumerate(frames):
                ci, c = (n // CHUNK) % 2, n % CHUNK
                out_f = stag_view(ci, c)  # [P,4,4,q]

                kind = fr["kind"]
                if kind == "const":
                    l_ap = None  # products read the table directly
                else:
                    l_t = lpool.tile([P, 16, q], F32, tag="L")
                    l_r = l_t[:].rearrange("p (k j) q -> p k j q", j=4)
                    ub = u_t[:, n, :].unsqueeze(1).unsqueeze(2) \
                        .broadcast_to([P, ni, 3, q])
                    if kind == "rev":
                        wb = w_t[:, n, :].unsqueeze(1).unsqueeze(2) \
                            .broadcast_to([P, ni, 3, q])
                        m_b = mpool.tile([P, ni, 3, q], F32, tag="mB")
                        m_c = mpool.tile([P, ni, 3, q], F32, tag="mC")
                        nc.gpsimd.tensor_mul(m_b[:], ub, tab_b(n, 1, ni, 3))
                        q4 = q // 4
                        wb4 = w_t[:, n, :q4].unsqueeze(1).unsqueeze(2) \
                            .broadcast_to([P, ni, 3, q4])
                        c4 = tab(n, 2)[:, :ni, :3].unsqueeze(3) \
                            .broadcast_to([P, ni, 3, q4])
                        nc.vector.tensor_mul(m_c[:][:, :, :, :q4], wb4, c4)
                        wbr = w_t[:, n, q4:].unsqueeze(1).unsqueeze(2) \
                            .broadcast_to([P, ni, 3, q - q4])
                        cr = tab(n, 2)[:, :ni, :3].unsqueeze(3) \
                            .broadcast_to([P, ni, 3, q - q4])
                        nc.gpsimd.tensor_mul(m_c[:][:, :, :, q4:], wbr, cr)
                        lr_s = mpool.tile([P, ni, 3, q], F32, tag="lrs")
                        nc.gpsimd.tensor_add(lr_s[:], m_b[:], m_c[:])
                        # L rotation block = sum + A'
                        nc.gpsimd.tensor_add(l_r[:, :ni, :3, :], lr_s[:],
                                             tab_b(n, 0, ni, 3))
                        if fr.get("tcol") is None:
                            # non-affine: materialize L t-col (A' col 3)
                            a_col3 = tab(n, 0)[:, :ni, 3].unsqueeze(2) \
                                .broadcast_to([P, ni, q])
                            nc.scalar.copy(l_r[:, :ni, 3, :], a_col3)
                        # else: t-col as immediates in the STT chain
                    else:  # prismatic: L = A + u*B ; B nonzero only in col 3
                        m_b = mpool.tile([P, ni, 1, q], F32, tag="mB")
                        ub1 = u_t[:, n, :].unsqueeze(1).unsqueeze(2) \
                            .broadcast_to([P, ni, 1, q])
                        nc.gpsimd.tensor_mul(
                            m_b[:], ub1,
                            tab(n, 1)[:, :ni, 3:4].unsqueeze(3)
                            .broadcast_to([P, ni, 1, q]))
                        nc.vector.tensor_add(
                            l_r[:, :ni, 3:4, :], m_b[:],
                            tab(n, 0)[:, :ni, 3:4].unsqueeze(3)
                            .broadcast_to([P, ni, 1, q]))
                        # rotation block is constant = A
                        a_rot = tab(n, 0)[:, :ni, :3].unsqueeze(3) \
                            .broadcast_to([P, ni, 3, q])
                        nc.scalar.copy(l_r[:, :ni, :3, :], a_rot)
                    l_ap = l_r

                def lrow(k):
                    # L row k broadcast over i: [P, ni, 4, q]
                    if l_ap is not None:
                        return l_ap[:, k, :, :].unsqueeze(1) \
                            .broadcast_to([P, ni, 4, q])
                    return tab(n, 0)[:, k, :].unsqueeze(1).unsqueeze(3) \
                        .broadcast_to([P, ni, 4, q])

                if prev is None:
                    # W_0 = L_0 : write directly into staging
                    if l_ap is not None:
                        if fr.get("tcol") is not None:
                            # L t-col not materialized; copy rot block +
                            # fill t-col from the table constants
                            nc.vector.tensor_copy(out_f[:, :ni, :3, :],
                                                  l_ap[:, :ni, :3, :])
                            nc.scalar.copy(
                                out_f[:, :ni, 3, :],
                                tab(n, 0)[:, :ni, 3].unsqueeze(2)
                                .broadcast_to([P, ni, q]))
                        else:
                            nc.vector.tensor_copy(out_f[:, :ni, :, :],
                                                  l_ap[:, :ni, :, :])
                    else:
                        nc.scalar.copy(
                            out_f[:, :ni, :, :],
                            tab(n, 0)[:, :ni, :].unsqueeze(3)
                            .broadcast_to([P, ni, 4, q]))
                else:
                    w_v = stag_view(*prev)  # [P,4,4,q] previous transform
                    tcol = fr.get("tcol")
                    nj = 3 if (nk == 3 and tcol is not None) else 4

                    def wcol(k):
                        return w_v[:, :ni, k, :].unsqueeze(2) \
                            .broadcast_to([P, ni, nj, q])

                    def lrowj(k):
                        if l_ap is not None:
                            return l_ap[:, k, :nj, :].unsqueeze(1) \
                                .broadcast_to([P, ni, nj, q])
                        return tab(n, 0)[:, k, :nj].unsqueeze(1).unsqueeze(3) \
                            .broadcast_to([P, ni, nj, q])

                    p0 = mpool.tile([P, ni, 4, q], F32, tag="p0")
                    p1 = mpool.tile([P, ni, 4, q], F32, tag="p1")
                    p0v = p0[:][:, :, :nj, :]
                    p1v = p1[:][:, :, :nj, :]
                    nc.vector.tensor_mul(p0v, wcol(0), lrowj(0))
                    nc.vector.tensor_mul(p1v, wcol(1), lrowj(1))
                    nc.vector.tensor_add(p0v, p0v, p1v)
                    nc.vector.tensor_mul(p1v, wcol(2), lrowj(2))
                    if nk == 3:
                        nc.vector.tensor_add(out_f[:, :ni, :nj, :], p0v, p1v)
                        if tcol is not None:
                            # t-col via const-immediate STT chain:
                            # out_j3 = W_i0*c0 + W_i3; += W_i1*c1; += W_i2*c2
                            o3 = out_f[:, :ni, 3, :]
                            nc.vector.scalar_tensor_tensor(
                                o3, w_v[:, :ni, 0, :], float(tcol[0]),
                                w_v[:, :ni, 3, :],
                                op0=op.mult, op1=op.add)
                            nc.vector.scalar_tensor_tensor(
                                o3, w_v[:, :ni, 1, :], float(tcol[1]), o3,
                                op0=op.mult, op1=op.add)
                            nc.vector.scalar_tensor_tensor(
                                o3, w_v[:, :ni, 2, :], float(tcol[2]), o3,
                                op0=op.mult, op1=op.add)
                        else:
                            # affine fix: out[:, i, 3] += W[:, i, 3]
                            nc.vector.tensor_add(out_f[:, :ni, 3, :],
                                                 out_f[:, :ni, 3, :],
                                                 w_v[:, :ni, 3, :])
                    else:
                        nc.vector.tensor_add(p0v, p0v, p1v)
                        nc.vector.tensor_mul(p1v, wcol(3), lrowj(3))
                        nc.vector.tensor_add(out_f[:, :ni, :, :], p0v, p1v)

                prev = (ci, c)

                # chunk complete -> DMA out
                if c == CHUNK - 1:
                    g = n // CHUNK
                    src = stags[ci][:].rearrange("p q c e -> p q (c e)")
                    dst = out_v[:, :, g * CHUNK:(g + 1) * CHUNK, :] \
                        .rearrange("p q c e -> p q (c e)")
                    nc.sync.dma_start(dst, src)

    nc.compile()
    return nc


def _get_program(b_core, dof, nf, affine, frames):
    key = (b_core, dof, nf, affine, os.environ.get("FK_REPS", "1"),
           tuple((f["kind"], f.get("src"), f.get("mult"), f.get("off"),
                  f.get("tcol")) for f in frames))
    prog = _program_cache.get(key)
    if prog is None:
        prog = _build_program(b_core, dof, nf, affine, frames)
        _program_cache[key] = prog
    return prog


# --------------------------------------------------------------------------
# Entry point
# --------------------------------------------------------------------------

def kernel(joint_angles, all_axes, all_origins, mimic_multipliers,
           mimic_offsets, ctrlable_indices, mimic_dst_indices,
           mimic_src_indices, joint_types):
    global last_results

    theta = np.ascontiguousarray(np.asarray(joint_angles, dtype=np.float32))
    batch, dof = theta.shape
    nf = np.asarray(all_axes).shape[0]

    affine, frames, tables = _frame_specs(
        all_axes, all_origins, mimic_multipliers, mimic_offsets,
        ctrlable_indices, mimic_dst_indices, mimic_src_indices, joint_types)

    n_cores = N_CORES
    assert batch % n_cores == 0
    b_core = batch // n_cores

    nc = _get_program(b_core, dof, nf, affine, frames)

    tables_rep = np.ascontiguousarray(
        np.broadcast_to(tables[None], (P, nf, 4, 16)).astype(np.float32))
    global last_tables_rep
    last_tables_rep = tables_rep

    in_maps = []
    for i in range(n_cores):
        in_maps.append({
            "theta": np.ascontiguousarray(theta[i * b_core:(i + 1) * b_core]),
            "tables": tables_rep,
        })

    res = run_bass_kernel_spmd(nc, in_maps, core_ids=list(range(n_cores)))
    last_results = res

    out = np.concatenate([res.results[i]["out"] for i in range(n_cores)], axis=0)
    return out.reshape(batch, nf, 4, 4)



# revision 16
# speedup vs baseline: 7.6226x; 7.6226x over previous
"""Trainium2 Bass kernel: batched forward kinematics (nn_DiffKin), v2.

Computes, for each batch element b and frame n:
    W[b, n] = prod_{i<=n} ( O_i @ M_i(angle_i(b)) )        (4x4 transforms)

v2 architecture (vs v1 which did L-build on GpSimd and an fp32 chain on DVE):
  * fp16 end-to-end on device (validated on host: rel_l2 ~2.0e-3 vs the
    fp64 reference; harness gate is 2e-2). Host upconverts to fp32.
  * Coefficient planes built s-major: theta is transposed on the (otherwise
    idle) TensorEngine into a [125, B_core] coefficient tile whose rows are
    [u-rows(61 rev) | pri-x(2) | w-rows(61) | ones]; ScalarE then applies
    Sin / Abs+Sin with per-partition (scale, bias) in 3 big ops.
  * Per-frame local transforms L_n = A'_n + sin(x)B_n + cos(x)(-C_n) for ALL
    frames come from PE matmuls: lhsT = coef column [125, 128] (stationary,
    one load per batch column), rhs = host-built block-sparse table
    [125, 64*12] fp16, split into 4 frame-groups of 16 so the chain can
    start after group 0. PSUM results are evacuated to an SBUF L table
    [P, nf, 12, q] fp16 by ScalarE.
  * The sequential chain W_n = W_{n-1} @ L_n runs on DVE (+ optional GpSimd
    q-slice) in fp16: 5 tensor_tensor ops on the 3x3 rotation block (2x DVE
    mode) + 3 scalar_tensor_tensor ops for the (constant-per-frame) t-column.
  * Output staged fp16, DMA'd out as fp16 (halves DMA bytes); host astype.
"""

import os
import sys

import numpy as np

for _p in ("/opt/trn_rl_repo", "/root/.axon_site/_ro/trn_rl_repo"):
    if os.path.isdir(_p) and _p not in sys.path:
        sys.path.append(_p)

import concourse.bass as bass  # noqa: E402
import concourse.tile as tile  # noqa: E402
from concourse import bacc, masks, mybir  # noqa: E402
from concourse.bass_utils import run_bass_kernel_spmd  # noqa: E402

F32 = mybir.dt.float32
F16 = mybir.dt.float16
AF = mybir.ActivationFunctionType
OP = mybir.AluOpType

N_CORES = 8
P = 128          # SBUF partitions
CHUNK = 8        # frames per output staging chunk
FG = 16          # frames per matmul group
QPAIR = 2        # batch-columns per PSUM evac group
QS = 64          # chain q-split: DVE gets [0:QS], GpSimd [QS:64]

last_results = None
last_in_maps = None
_program_cache = {}


def _skew(a):
    x, y, z = a
    return np.array([[0.0, -z, y], [z, 0.0, -x], [-y, x, 0.0]], dtype=np.float64)


# --------------------------------------------------------------------------
# Host-side specialization
# --------------------------------------------------------------------------

def _frame_specs(all_axes, all_origins, mimic_multipliers, mimic_offsets,
                 ctrlable_indices, mimic_dst_indices, mimic_src_indices,
                 joint_types):
    """Fold structural inputs into per-frame specs.

    Returns frames: list of dicts per frame:
        kind: 'rev' | 'pri' | 'const'
        src, mult, off  (var kinds; angle_n(b) = mult*theta[b,src] + off)
        A, B, C : constant 4x4 float64 blocks (B/C only for var kinds)
    """
    axes = np.asarray(all_axes, dtype=np.float64)
    origins = np.asarray(all_origins, dtype=np.float64)
    nf = origins.shape[0]
    types = np.asarray(joint_types).astype(np.int64)
    ctrl = np.asarray(ctrlable_indices).astype(np.int64)
    mdst = np.asarray(mimic_dst_indices).astype(np.int64)
    msrc = np.asarray(mimic_src_indices).astype(np.int64)
    mmul = np.asarray(mimic_multipliers, dtype=np.float64)
    moff = np.asarray(mimic_offsets, dtype=np.float64)

    bottom = origins[:, 3, :]
    affine = bool(np.all(np.abs(bottom - np.array([0.0, 0.0, 0.0, 1.0])) < 1e-6))
    assert affine, "v2 kernel requires affine origins"

    src = [None] * nf
    mult = [0.0] * nf
    off = [0.0] * nf
    for j, ci in enumerate(ctrl):
        src[int(ci)] = j
        mult[int(ci)] = 1.0
        off[int(ci)] = 0.0
    pre = (list(src), list(mult), list(off))
    for d, s, m, o in zip(mdst, msrc, mmul, moff):
        d, s = int(d), int(s)
        if pre[0][s] is not None:
            src[d] = pre[0][s]
            mult[d] = float(m) * pre[1][s]
            off[d] = float(m) * pre[2][s] + float(o)
        else:
            src[d] = None
            mult[d] = 0.0
            off[d] = float(o)

    frames = []
    for n in range(nf):
        O4 = origins[n]
        t = int(types[n])
        if t == 1:
            r = float(np.linalg.norm(axes[n]))
            if r < 1e-20 or src[n] is None:
                # degenerate or constant-angle revolute -> constant frame
                if src[n] is None and r >= 1e-20:
                    K4 = np.zeros((4, 4))
                    K4[:3, :3] = _skew(axes[n] / r)
                    a = r * off[n]
                    M = (O4 + np.sin(a) * (O4 @ K4)
                         + (1.0 - np.cos(a)) * (O4 @ K4 @ K4))
                else:
                    M = O4
                frames.append(dict(kind="const", A=M))
            else:
                K4 = np.zeros((4, 4))
                K4[:3, :3] = _skew(axes[n] / r)
                frames.append(dict(kind="rev", src=src[n],
                                   mult=r * mult[n], off=r * off[n],
                                   A=O4, B=O4 @ K4, C=O4 @ K4 @ K4))
        elif t == 2:
            T4 = np.zeros((4, 4))
            T4[:3, 3] = axes[n]
            B = O4 @ T4
            if src[n] is None:
                frames.append(dict(kind="const", A=O4 + off[n] * B))
            else:
                frames.append(dict(kind="pri", src=src[n],
                                   mult=mult[n], off=off[n], A=O4, B=B))
        else:
            frames.append(dict(kind="const", A=O4))
    return frames


def _host_spec(frames):
    """Row layout, table, per-row consts, chain immediates."""
    nf = len(frames)
    rev = [n for n, f in enumerate(frames) if f["kind"] == "rev"]
    pri = [n for n, f in enumerate(frames) if f["kind"] == "pri"]
    nu, npri = len(rev), len(pri)
    # rows: [0..nu) rev-u | [nu..nu+npri) pri-x | pad | [w0..w0+nu) rev-w
    #       | ones row.  w0 is 32-aligned: engine ops on the w rows must
    #       start at a partition base that is a multiple of 32.
    w0 = ((nu + npri + 31) // 32) * 32
    ones_row = w0 + nu
    K = ones_row + 1
    assert K <= P

    srccol = [None] * (ones_row)       # theta column feeding each x row
    mult_arr = np.ones(P, np.float64)
    off_arr = np.zeros(P, np.float64)
    u_row = {}
    w_row = {}
    for i, n in enumerate(rev):
        f = frames[n]
        u_row[n] = i
        w_row[n] = w0 + i
        srccol[i] = f["src"]
        srccol[w0 + i] = f["src"]
        mult_arr[i] = mult_arr[w0 + i] = f["mult"]
        off_arr[i] = off_arr[w0 + i] = f["off"]
    for i, n in enumerate(pri):
        u_row[n] = nu + i
        srccol[nu + i] = frames[n]["src"]
        # pri rows get (mult, off) applied batch-major before the transpose

    table = np.zeros((P, nf * 12), np.float64)
    tcols = [None] * nf               # chain t-col immediates (rev/const)
    for n, f in enumerate(frames):
        cols = slice(12 * n, 12 * n + 12)
        if f["kind"] == "rev":
            Ap = f["A"] + f["C"]
            table[ones_row, cols] = Ap[:3, :].reshape(-1)
            table[u_row[n], cols] = f["B"][:3, :].reshape(-1)
            table[w_row[n], cols] = (-f["C"])[:3, :].reshape(-1)
            tcols[n] = tuple(float(v) for v in Ap[:3, 3])
        elif f["kind"] == "pri":
            table[ones_row, cols] = f["A"][:3, :].reshape(-1)
            table[u_row[n], cols] = f["B"][:3, :].reshape(-1)
        else:
            table[ones_row, cols] = f["A"][:3, :].reshape(-1)
            tcols[n] = tuple(float(v) for v in f["A"][:3, 3])

    # contiguous copy runs for theta_dup build: (dst0, src0, len);
    # rows with srccol None (pri rows, pad rows) are handled separately.
    runs = []
    zero_rows = [r for r in range(nu + npri, w0)]   # pad rows -> memset 0
    r = 0
    while r < ones_row:
        if srccol[r] is None:
            r += 1
            continue
        s = srccol[r]
        ln = 1
        while r + ln < ones_row and srccol[r + ln] == s + ln:
            ln += 1
        runs.append((r, s, ln))
        r += ln

    pri_rows = [(nu + i, frames[n]["src"], frames[n]["mult"], frames[n]["off"])
                for i, n in enumerate(pri)]
    consts = np.zeros((P, 4), np.float32)
    consts[:, 0] = mult_arr
    consts[:, 1] = off_arr
    consts[:, 2] = np.pi / 2.0
    return dict(K=K, nu=nu, npri=npri, w0=w0, ones_row=ones_row,
                runs=runs, pri_rows=pri_rows, zero_rows=zero_rows,
                consts=consts, table=table.astype(np.float16), tcols=tcols,
                kinds=[f["kind"] for f in frames])


# --------------------------------------------------------------------------
# Device program
# --------------------------------------------------------------------------

def _build_program(b_core, dof, nf, spec):
    assert b_core % P == 0
    q = b_core // P
    K = spec["K"]
    nu, w0, ones_row = spec["nu"], spec["w0"], spec["ones_row"]
    ngrp = (nf + FG - 1) // FG
    assert nf % FG == 0 and nf % CHUNK == 0
    tcols = spec["tcols"]
    kinds = spec["kinds"]
    reps = int(os.environ.get("FK_REPS", "1"))

    nc = bacc.Bacc("TRN2", target_bir_lowering=False, debug=False)

    theta_d = nc.dram_tensor("theta", [b_core, dof], F32,
                             kind="ExternalInput").ap()
    table_d = nc.dram_tensor("table", [P, nf * 12], F16,
                             kind="ExternalInput").ap()
    consts_d = nc.dram_tensor("consts", [P, 4], F32,
                              kind="ExternalInput").ap()
    # output layout [p, n, e, q] (q innermost): whole staging chunks DMA out
    # as one contiguous 16KB run per partition; host permutes to [b, n, e].
    out_d = nc.dram_tensor("out", [P, nf * 16 * q], F16,
                           kind="ExternalOutput").ap()

    theta_v = theta_d.rearrange("(p q) d -> p q d", p=P)

    from contextlib import ExitStack

    with tile.TileContext(nc) as tc, ExitStack() as ctx:
        pool = ctx.enter_context(tc.tile_pool(name="persist", bufs=1))
        tpp = ctx.enter_context(tc.tile_pool(name="tp_psum", bufs=4,
                                             space=bass.MemorySpace.PSUM))
        mmp = ctx.enter_context(tc.tile_pool(name="mm_psum", bufs=4,
                                             space=bass.MemorySpace.PSUM))
        mpool = ctx.enter_context(tc.tile_pool(name="mpool", bufs=4))

        theta_t = pool.tile([P, q, dof], F32)
        nc.sync.dma_start(theta_t[:], theta_v)
        table_t = pool.tile([P, nf * 12], F16)
        nc.sync.dma_start(table_t[:], table_d)
        consts_t = pool.tile([P, 4], F32)
        nc.sync.dma_start(consts_t[:], consts_d)

        ident = pool.tile([P, P], F16)
        masks.make_identity(nc, ident[:])

        theta_dup = pool.tile([P, q, K], F16)
        coef_t = pool.tile([P, q, P], F16)     # [K rows, qq, 128]
        l_t = pool.tile([P, nf, 12, q], F16)

        stags = [pool.tile([P, CHUNK, 16, q], F16, tag=f"stag{i}",
                           name=f"stag{i}") for i in range(2)]
        for st in stags:
            nc.vector.memset(st[:, :, 12:15, :], 0.0)
            nc.vector.memset(st[:, :, 15, :], 1.0)

        for _rep in range(reps):
            # ---- theta_dup: batch-major x columns (dup for w rows) -------
            for dst0, src0, ln in spec["runs"]:
                nc.scalar.copy(theta_dup[:, :, dst0:dst0 + ln],
                               theta_t[:, :, src0:src0 + ln])
            for r, s, m, o in spec["pri_rows"]:
                nc.vector.tensor_scalar(theta_dup[:, :, r], theta_t[:, :, s],
                                        float(m), float(o),
                                        op0=OP.mult, op1=OP.add)
            for r0 in spec["zero_rows"]:
                nc.vector.memset(theta_dup[:, :, r0], 0.0)
            nc.vector.memset(theta_dup[:, :, ones_row], 1.0)

            # ---- transpose to s-major coef tile --------------------------
            for qq in range(q):
                ps = tpp.tile([K, P], F16, tag="tp")
                nc.tensor.transpose(ps[:], theta_dup[:, qq, :], ident[:])
                nc.scalar.copy(coef_t[:K, qq, :], ps[:])

            # ---- sin / cos via per-partition scale+bias ------------------
            # u rows: sin(mult*theta + off); w rows: sin(pi/2 - |mult*theta+off|)
            cview = coef_t[:].rearrange("p qq m -> p (qq m)")
            nc.scalar.activation(cview[0:nu], cview[0:nu], AF.Sin,
                                 bias=consts_t[0:nu, 1:2],
                                 scale=consts_t[0:nu, 0:1])
            nc.scalar.activation(cview[w0:w0 + nu], cview[w0:w0 + nu], AF.Abs,
                                 bias=consts_t[w0:w0 + nu, 1:2],
                                 scale=consts_t[w0:w0 + nu, 0:1])
            nc.scalar.activation(cview[w0:w0 + nu], cview[w0:w0 + nu], AF.Sin,
                                 bias=consts_t[w0:w0 + nu, 2:3], scale=-1.0)

            # ---- L via PE: [K,128] coef col x [K, 192] table group -------
            for g in range(ngrp):
                gcol = slice(12 * FG * g, 12 * FG * (g + 1))
                for qq0 in range(0, q, QPAIR):
                    pmm = mmp.tile([P, QPAIR, 12 * FG], F32, tag="mm")
                    for j in range(QPAIR):
                        nc.tensor.matmul(pmm[:, j, :],
                                         coef_t[:K, qq0 + j, :],
                                         table_t[:K, gcol],
                                         start=True, stop=True)
                    # evac PSUM -> L fp16 [P, FG, 12, QPAIR]
                    dst = l_t[:, FG * g:FG * (g + 1), :, qq0:qq0 + QPAIR]
                    src = pmm[:].rearrange("p j (f e) -> p j f e", e=12) \
                        .transpose([0, 2, 3, 1])
                    nc.scalar.copy(dst, src)

            # ---- chain -------------------------------------------------
            def lrow(n, k, nj):
                # L_n row k broadcast over i: [P, 3, nj, q]
                return l_t[:, n, 4 * k:4 * k + nj, :].unsqueeze(1) \
                    .broadcast_to([P, 3, nj, q])

            def stag_view(ci, c):
                return stags[ci][:, c, :, :] \
                    .rearrange("p (i j) q -> p i j q", j=4)

            def tt(op, out, a, b):
                if QS >= q:
                    getattr(nc.vector, op)(out, a, b)
                else:
                    getattr(nc.vector, op)(out[..., :QS], a[..., :QS],
                                           b[..., :QS])
                    getattr(nc.gpsimd, op)(out[..., QS:], a[..., QS:],
                                           b[..., QS:])

            def stt(out, in0, s, in1):
                if QS >= q:
                    nc.vector.scalar_tensor_tensor(out, in0, s, in1,
                                                   op0=OP.mult, op1=OP.add)
                else:
                    nc.vector.scalar_tensor_tensor(
                        out[..., :QS], in0[..., :QS], s, in1[..., :QS],
                        op0=OP.mult, op1=OP.add)
                    nc.gpsimd.scalar_tensor_tensor(
                        out[..., QS:], in0[..., QS:], s, in1[..., QS:],
                        op0=OP.mult, op1=OP.add)

            prev = None
            for n in range(nf):
                ci, c = (n // CHUNK) % 2, n % CHUNK
                out_f = stag_view(ci, c)     # [P, 4, 4, q]

                if prev is None:
                    nc.vector.tensor_copy(
                        out_f[:, :3, :, :],
                        l_t[:, n, :, :].rearrange("p (k j) q -> p k j q", j=4))
                    prev = (ci, c)
                    continue

                w_v = stag_view(*prev)
                if kinds[n] != "pri":
                    nj = 3

                    def wcol(k):
                        return w_v[:, :3, k, :].unsqueeze(2) \
                            .broadcast_to([P, 3, nj, q])

                    p0 = mpool.tile([P, 3, 4, q], F16, tag="p0")
                    p1 = mpool.tile([P, 3, 4, q], F16, tag="p1")
                    p0v = p0[:][:, :, :nj, :]
                    p1v = p1[:][:, :, :nj, :]
                    tt("tensor_mul", p0v, wcol(0), lrow(n, 0, nj))
                    tt("tensor_mul", p1v, wcol(1), lrow(n, 1, nj))
                    tt("tensor_add", p0v, p0v, p1v)
                    tt("tensor_mul", p1v, wcol(2), lrow(n, 2, nj))
                    tt("tensor_add", out_f[:, :3, :3, :], p0v, p1v)
                    tc_ = tcols[n]
                    o3 = out_f[:, :3, 3, :]
                    stt(o3, w_v[:, :3, 0, :], float(tc_[0]), w_v[:, :3, 3, :])
                    stt(o3, w_v[:, :3, 1, :], float(tc_[1]), o3)
                    stt(o3, w_v[:, :3, 2, :], float(tc_[2]), o3)
                else:
                    nj = 4

                    def wcol4(k):
                        return w_v[:, :3, k, :].unsqueeze(2) \
                            .broadcast_to([P, 3, 4, q])

                    p0 = mpool.tile([P, 3, 4, q], F16, tag="p0")
                    p1 = mpool.tile([P, 3, 4, q], F16, tag="p1")
                    tt("tensor_mul", p0[:], wcol4(0), lrow(n, 0, 4))
                    tt("tensor_mul", p1[:], wcol4(1), lrow(n, 1, 4))
                    tt("tensor_add", p0[:], p0[:], p1[:])
                    tt("tensor_mul", p1[:], wcol4(2), lrow(n, 2, 4))
                    tt("tensor_add", out_f[:, :3, :, :], p0[:], p1[:])
                    tt("tensor_add", out_f[:, :3, 3, :],
                       out_f[:, :3, 3, :], w_v[:, :3, 3, :])

                prev = (ci, c)

                if c == CHUNK - 1:
                    g = n // CHUNK
                    blk = CHUNK * 16 * q
                    src = stags[ci][:].rearrange("p c e q -> p (c e q)")
                    dst = out_d[:, g * blk:(g + 1) * blk]
                    nc.sync.dma_start(dst, src)

    nc.compile()
    return nc


def _get_program(b_core, dof, nf, spec_key, spec):
    key = (b_core, dof, nf, os.environ.get("FK_REPS", "1"), QS, spec_key)
    prog = _program_cache.get(key)
    if prog is None:
        prog = _build_program(b_core, dof, nf, spec)
        _program_cache[key] = prog
    return prog


# --------------------------------------------------------------------------
# Entry point
# --------------------------------------------------------------------------

def kernel(joint_angles, all_axes, all_origins, mimic_multipliers,
           mimic_offsets, ctrlable_indices, mimic_dst_indices,
           mimic_src_indices, joint_types):
    global last_results, last_in_maps

    theta = np.ascontiguousarray(np.asarray(joint_angles, dtype=np.float32))
    batch, dof = theta.shape
    nf = np.asarray(all_axes).shape[0]

    frames = _frame_specs(
        all_axes, all_origins, mimic_multipliers, mimic_offsets,
        ctrlable_indices, mimic_dst_indices, mimic_src_indices, joint_types)
    spec = _host_spec(frames)
    spec_key = (spec["K"], spec["nu"], spec["npri"],
                tuple(spec["kinds"]), tuple(spec["runs"]),
                tuple(spec["pri_rows"]),
                spec["table"].tobytes(), spec["consts"].tobytes(),
                tuple(t if t is None else tuple(t) for t in spec["tcols"]))

    n_cores = N_CORES
    assert batch % n_cores == 0
    b_core = batch // n_cores

    nc = _get_program(b_core, dof, nf, spec_key, spec)

    in_maps = []
    for i in range(n_cores):
        in_maps.append({
            "theta": np.ascontiguousarray(theta[i * b_core:(i + 1) * b_core]),
            "table": np.ascontiguousarray(spec["table"]),
            "consts": np.ascontiguousarray(spec["consts"]),
        })
    last_in_maps = in_maps

    res = run_bass_kernel_spmd(nc, in_maps, core_ids=list(range(n_cores)))
    last_results = res

    q = b_core // P
    parts = []
    for i in range(n_cores):
        o = res.results[i]["out"].reshape(P, nf, 16, q)
        # [p, n, e, q] -> [p, q, n, e]; core-local batch b = p*q + qq
        parts.append(np.transpose(o, (0, 3, 1, 2)).reshape(b_core, nf, 16))
    out = np.concatenate(parts, axis=0)
    return out.astype(np.float32).reshape(batch, nf, 4, 4)
